# revision 12
# baseline (speedup 1.0000x reference)
"""Trainium2 Bass kernel for a Bahdanau-attention GRU decoder.

Reference computation (T=512, B=128, I=H=512, O=12, L=max_labels=16):
    s0 = tanh(x[0] @ ws);  out0 = s0 @ fc_w + fc_b
    U  = einsum('tbi,ih->tbh', x, ua)            # precomputed once
    per step:
        e  = einsum('tbh,h->tb', tanh(s @ wa + U), va)
        a  = softmax(e, axis=t)
        c  = einsum('tb,tbi->bi', a, x)
        r  = sigmoid(out @ wr + s @ ur + c @ cr)
        z  = sigmoid(out @ wz + s @ uz + c @ cz)
        sh = tanh(out @ w0 + (r*s) @ u0 + c @ c0)
        s  = (1-z)*s + z*sh;  out = s @ fc_w + fc_b
    returns [B, L, O]

Sharding: data-parallel over batch B across 8 cores (BL=16 per core), all
weights replicated; no collectives.  Per core, x (fp16, [i,(b,t)] natural
tiles) and U (fp16, [h-part, t, b]) are SBUF-resident so the recurrence never
touches HBM.

Per-step engine split:
  DVE : V = U + broadcast(s@wa^T)   (fp16 tensor_tensor, 2x mode, b-innermost)
  ACT : tanh(V) in-place on [128, 256*16] slabs; exp for softmax; gate tanh
        (sigmoid is computed as 0.5*tanh(x/2)+0.5 to stay in one ACT table set)
  PE  : e-dot via constant "va-selector" lhsT [128,16] (column b = va chunk)
        accumulating all b into one PSUM bank as e[b, t]; context matvecs;
        gate matmuls in natural orientation (lhsT = small transposed states);
        128x128 transposes for state/layout changes.
"""

import numpy as np
from contextlib import ExitStack

import concourse.bass as bass
import concourse.mybir as mybir
import concourse.tile as tile
from concourse import bacc
from concourse.bass_utils import run_bass_kernel_spmd
from concourse.masks import make_identity

F32 = mybir.dt.float32
F16 = mybir.dt.float16
AF = mybir.ActivationFunctionType
ALU = mybir.AluOpType
AX = mybir.AxisListType

T, B, I, H, O = 512, 128, 512, 512, 12
P = 128
NCORES = 8
BL = B // NCORES        # 16 batches per core
HC = H // P             # 4 h-chunks
IC = I // P             # 4 i-chunks
TC = T // P             # 4 t-chunks
NTH = 2                 # t-halves for the attention slabs
THL = T // NTH          # 256

WNAMES = ["w0", "wz", "wr", "ws", "wa", "ua", "va", "u0", "uz", "ur",
          "c0", "cz", "cr", "fc_w", "fc_b"]


def _load_weight_pkh(nc, pool, wname, ap, kc, cast_pool, dtype=F16):
    """DRAM [K, H] fp32 -> SBUF [P, kc, H] in `dtype` (cast via DVE copy)."""
    w16 = pool.tile([P, kc, H], dtype, name=f"{wname}_sb")
    ap3 = ap.rearrange("(c p) h -> p c h", p=P)
    for c in range(kc):
        tmp = cast_pool.tile([P, H], F32, tag="wload", name=f"{wname}_f32tmp")
        nc.sync.dma_start(tmp[:], ap3[:, c, :])
        nc.vector.tensor_copy(w16[:, c, :], tmp[:])
    return w16


def _transpose_small(nc, psum_pool, ident16, src, dst, nchunk, tagp="tp"):
    """src [BL, nchunk*128] f16 SBUF  ->  dst [P, nchunk, BL] f16 SBUF
    via nchunk PE transposes + 1 copy."""
    ps = psum_pool.tile([P, nchunk, BL], F16, tag=tagp, name="tps")
    for c in range(nchunk):
        nc.tensor.transpose(ps[:, c, :], src[:, c * P:(c + 1) * P],
                            ident16[:BL, :BL])
    nc.vector.tensor_copy(dst[:], ps[:])


def _build_decoder(ctx: ExitStack, tc_: tile.TileContext, L: int, io: dict,
                   debug: bool = False):
    nc = tc_.nc
    x, out = io["x"], io["out"]

    const = ctx.enter_context(tc_.tile_pool(name="const", bufs=1))
    big = ctx.enter_context(tc_.tile_pool(name="big", bufs=1))

    ident16 = const.tile([P, P], F16)
    make_identity(nc, ident16[:])
    ident32 = const.tile([P, P], F32)
    make_identity(nc, ident32[:])

    # ---------------- persistent SBUF tensors ----------------
    x_nat = big.tile([P, BL, TC, I], F16)    # x[t%128, b, t//128, i]   64KB/par
    U_sb = big.tile([P, HC, T, BL], F16)     # U[h%128, h//128, t, b]   64KB/par

    # ---------------- state tiles (ping-pong via bufs=2 pools) ----------------
    state = ctx.enter_context(tc_.tile_pool(name="state", bufs=2))

    s_nat = state.tile([BL, H], F32, tag="s", name="s0_nat")
    sT_f32 = state.tile([P, HC, BL], F32, tag="sT32", name="s0T_f32")
    sT_f16 = state.tile([P, HC, BL], F16, tag="sT16", name="s0T_f16")
    out_nat = state.tile([BL, O], F32, tag="out", name="out0_nat")
    outT_f16 = state.tile([P, BL], F16, tag="outT", name="out0T_f16")

    # ---------------- setup: weights, load x, transpose, U = x @ ua, s0 ------
    with tc_.tile_pool(name="setup", bufs=2) as stp, \
         tc_.tile_pool(name="setup1", bufs=1) as stp1, \
         tc_.tile_pool(name="wcast", bufs=2) as wcast, \
         tc_.tile_pool(name="stpsA", bufs=2, space="PSUM") as stpsA, \
         tc_.tile_pool(name="stpsB", bufs=2, space="PSUM") as stpsB, \
         tc_.tile_pool(name="stpsC", bufs=1, space="PSUM") as stpsC:

        wa_sb = _load_weight_pkh(nc, const, "wa", io["wa"], HC, wcast)
        ur_sb = _load_weight_pkh(nc, const, "ur", io["ur"], HC, wcast)
        uz_sb = _load_weight_pkh(nc, const, "uz", io["uz"], HC, wcast)
        u0_sb = _load_weight_pkh(nc, const, "u0", io["u0"], HC, wcast)
        cr_sb = _load_weight_pkh(nc, const, "cr", io["cr"], IC, wcast)
        cz_sb = _load_weight_pkh(nc, const, "cz", io["cz"], IC, wcast)
        c0_sb = _load_weight_pkh(nc, const, "c0", io["c0"], IC, wcast)

        # [O, H] gate input weights, zero-padded to K=128 partitions
        # (K<128 matmuls are unreliable: the PE contracts over the full
        # partition range, so unused partitions must be zero)
        wsmall = {}
        for nm in ("wr", "wz", "w0"):
            tmp = wcast.tile([O, H], F32, tag="wsload", name=f"{nm}_f32tmp")
            nc.sync.dma_start(tmp[:], io[nm])
            w16 = const.tile([P, H], F16, name=f"{nm}_sb")
            nc.vector.memset(w16[:], 0.0)
            nc.vector.tensor_copy(w16[:O, :], tmp[:])
            wsmall[nm] = w16

        # fc kept fp32 for output accuracy
        fcw_sb = const.tile([P, HC, O], F32)
        nc.sync.dma_start(fcw_sb[:],
                          io["fc_w"].rearrange("(c p) o -> p c o", p=P))
        fcb_sb = const.tile([BL, O], F32)
        nc.sync.dma_start(fcb_sb[:], io["fc_b"][None, :].to_broadcast((BL, O)))

        # va -> VaSel[p, hc, b, m] = va[hc*128+p] * (m == b)
        va_f32 = const.tile([P, HC], F32)
        nc.sync.dma_start(va_f32[:],
                          io["va"][:, 0].rearrange("(c p) -> p c", p=P))
        va_f16 = const.tile([P, HC], F16)
        nc.vector.tensor_copy(va_f16[:], va_f32[:])
        vasel = const.tile([P, HC, BL, BL], F16)
        nc.vector.memset(vasel[:], 0.0)
        for hc in range(HC):
            for b in range(BL):
                nc.vector.tensor_copy(vasel[:, hc, b, b:b + 1],
                                      va_f16[:, hc:hc + 1])
        # one-hot mask used to build the per-step context selector
        bsel = const.tile([P, BL, BL], F16)
        nc.vector.memset(bsel[:], 0.0)
        one_f16 = const.tile([P, 1], F16)
        nc.vector.memset(one_f16[:], 1.0)
        for b in range(BL):
            nc.vector.tensor_copy(bsel[:, b, b:b + 1], one_f16[:])

        ua_sb = _load_weight_pkh(nc, stp1, "ua", io["ua"], IC, wcast)
        ws_sb = _load_weight_pkh(nc, stp1, "ws", io["ws"], IC, wcast)

        GB = 2  # batches per transpose group
        for g in range(BL // GB):
            xT_g = stp.tile([P, IC, GB, T], F16, tag="xTg", name="xT_g")
            for bi in range(GB):
                b = g * GB + bi
                for t_ in range(TC):
                    xdma = stp.tile([P, I], F32, tag="xdma", name="xdma")
                    nc.sync.dma_start(xdma[:], x[t_ * P:(t_ + 1) * P, b, :])
                    nc.vector.tensor_copy(x_nat[:, b, t_, :], xdma[:])
                # transpose [t,i] tiles -> xT_g[i, t]
                for ic in range(IC):
                    tps = stpsA.tile([P, T], F16, tag="xtp", name="xtp")
                    for t_ in range(TC):
                        nc.tensor.transpose(
                            tps[:, t_ * P:(t_ + 1) * P],
                            x_nat[:, b, t_, ic * P:(ic + 1) * P], ident16[:])
                    nc.vector.tensor_copy(xT_g[:, ic, bi, :], tps[:])
                # U[:, hc, :, b] = sum_ic ua[ic]^T-chunk . xT
                for hc in range(HC):
                    ups = stpsB.tile([P, T], F32, tag="ups", name="ups")
                    for ic in range(IC):
                        nc.tensor.matmul(
                            ups[:], ua_sb[:, ic, hc * P:(hc + 1) * P],
                            xT_g[:, ic, bi, :],
                            start=(ic == 0), stop=(ic == IC - 1))
                    if hc % 2 == 0:
                        nc.vector.tensor_copy(U_sb[:, hc, :, b], ups[:])
                    else:
                        nc.scalar.copy(U_sb[:, hc, :, b], ups[:])

        # ---- s0 = tanh(x0 @ ws), out0 = s0 @ fc_w + fc_b ----
        x0_f32 = stp1.tile([BL, I], F32)
        nc.sync.dma_start(x0_f32[:], x[0, :, :])
        x0_f16 = stp1.tile([BL, I], F16)
        nc.vector.tensor_copy(x0_f16[:], x0_f32[:])
        x0T = stp1.tile([P, IC, BL], F16)
        _transpose_small(nc, stpsA, ident16, x0_f16, x0T, IC, tagp="xtp")

        s0T_ps = stpsC.tile([P, HC, BL], F32, name="s0T_ps")
        for hc in range(HC):
            for ic in range(IC):
                nc.tensor.matmul(
                    s0T_ps[:, hc, :], ws_sb[:, ic, hc * P:(hc + 1) * P],
                    x0T[:, ic, :], start=(ic == 0), stop=(ic == IC - 1))
        nc.scalar.activation(sT_f16[:], s0T_ps[:], AF.Tanh)
        nc.scalar.activation(sT_f32[:], s0T_ps[:], AF.Tanh)
        # s natural
        sps = stpsB.tile([BL, H], F32, tag="s0nat", name="s0nat_ps", bufs=1)
        for hc in range(HC):
            nc.tensor.transpose(sps[:, hc * P:(hc + 1) * P],
                                sT_f32[:, hc, :], ident32[:])
        nc.vector.tensor_copy(s_nat[:], sps[:])

    # ---------------- step-loop pools (opened after setup frees SBUF) -------
    work = ctx.enter_context(tc_.tile_pool(name="work", bufs=1))
    f16s = ctx.enter_context(tc_.tile_pool(name="f16s", bufs=1))
    vpool = ctx.enter_context(tc_.tile_pool(name="vpool", bufs=2))
    psA = ctx.enter_context(tc_.tile_pool(name="psA", bufs=1, space="PSUM"))
    psT = ctx.enter_context(tc_.tile_pool(name="psT", bufs=2, space="PSUM"))
    psG = ctx.enter_context(tc_.tile_pool(name="psG", bufs=2, space="PSUM"))
    psC = ctx.enter_context(tc_.tile_pool(name="psC", bufs=3, space="PSUM"))

    # out0
    ops = psT.tile([BL, O], F32, tag="tp", name="out0_ps")
    for kc in range(HC):
        nc.tensor.matmul(ops[:], sT_f32[:, kc, :], fcw_sb[:, kc, :],
                         start=(kc == 0), stop=(kc == HC - 1))
    nc.vector.tensor_tensor(out_nat[:], ops[:], fcb_sb[:], ALU.add)
    nc.sync.dma_start(out[0], out_nat[:])
    out_f16 = f16s.tile([BL, O], F16, tag="of16", name="out0_f16")
    nc.vector.tensor_copy(out_f16[:], out_nat[:])
    otp = psT.tile([O, BL], F16, tag="tp", name="out0T_ps")
    nc.tensor.transpose(otp[:], out_f16[:], ident16[:BL, :BL])
    nc.vector.memset(outT_f16[:], 0.0)
    nc.vector.tensor_copy(outT_f16[:O, :], otp[:])

    # ---------------- decode steps ----------------
    for k in range(1, L):
        # --- sWa = s @ wa (natural), then transposed fp16 copy ---
        swps = psT.tile([BL, H], F32, tag="tp", name="sw_ps")
        for kc in range(HC):
            nc.tensor.matmul(swps[:], sT_f16[:, kc, :], wa_sb[:, kc, :],
                             start=(kc == 0), stop=(kc == HC - 1))
        swa_f16 = f16s.tile([BL, H], F16, tag="swa", name="swa_f16")
        nc.vector.tensor_copy(swa_f16[:], swps[:])
        swaT = f16s.tile([P, HC, BL], F16, tag="swaT", name="swaT")
        _transpose_small(nc, psT, ident16, swa_f16, swaT, HC)

        # --- attention: V = tanh(U + swaT) ; e[b,t] accumulated on PE ---
        # NOTE: matmul start=True clears has_written for the WHOLE psum
        # bank, so each t-half's accumulation group must run contiguously
        # (th outer loop) -- interleaving the regions loses partial sums.
        e_ps = psA.tile([BL, T], F32, tag="e", name="e_ps")
        for th in range(NTH):
            for hc in range(HC):
                v = vpool.tile([P, THL, BL], F16, tag="v", name="vslab")
                nc.vector.tensor_tensor(
                    v[:], U_sb[:, hc, th * THL:(th + 1) * THL, :],
                    swaT[:, hc, None, :].to_broadcast((P, THL, BL)), ALU.add)
                nc.scalar.activation(v[:], v[:], AF.Tanh)
                for b in range(BL):
                    nc.tensor.matmul(
                        e_ps[:, th * THL:(th + 1) * THL],
                        vasel[:, hc, b, :], v[:, :, b],
                        start=(hc == 0 and b == 0),
                        stop=(hc == HC - 1 and b == BL - 1))

        # --- softmax over t (rows = b) ---
        emax = work.tile([BL, 1], F32, tag="emax", name="emax")
        nc.vector.tensor_reduce(emax[:], e_ps[:], axis=AX.X, op=ALU.max)
        emaxn = work.tile([BL, 1], F32, tag="emaxn", name="emaxn")
        nc.vector.tensor_scalar_mul(emaxn[:], emax[:], -1.0)
        a_f16 = f16s.tile([BL, T], F16, tag="a", name="a_f16")
        asum = work.tile([BL, 1], F32, tag="asum", name="asum")
        nc.scalar.activation(a_f16[:], e_ps[:], AF.Exp, bias=emaxn[:],
                             accum_out=asum[:])
        rsum = work.tile([BL, 1], F32, tag="rsum", name="rsum")
        nc.vector.reciprocal(rsum[:], asum[:])
        nc.vector.tensor_scalar_mul(a_f16[:], a_f16[:], rsum[:])
        aT = f16s.tile([P, TC, BL], F16, tag="aT", name="aT")
        _transpose_small(nc, psT, ident16, a_f16, aT, TC)

        # --- context c[b, i] = sum_t a[t,b] x[t,b,i] ---
        # aSel[p, tc, b, m] = aT[p, tc, b] * (m == b); then M=16 matmuls
        # accumulate every b into one [BL, I] psum tile.
        asel = f16s.tile([P, TC, BL, BL], F16, tag="asel", name="asel")
        nc.vector.tensor_tensor(
            asel[:], aT[:, :, :, None].to_broadcast((P, TC, BL, BL)),
            bsel[:, None, :, :].to_broadcast((P, TC, BL, BL)), ALU.mult)
        cps = psC.tile([BL, I], F32, tag="c", name="c_ps")
        for b in range(BL):
            for t_ in range(TC):
                nc.tensor.matmul(cps[:], asel[:, t_, b, :],
                                 x_nat[:, b, t_, :],
                                 start=(b == 0 and t_ == 0),
                                 stop=(b == BL - 1 and t_ == TC - 1))
        c_f16 = f16s.tile([BL, I], F16, tag="c", name="c_f16")
        nc.vector.tensor_copy(c_f16[:], cps[:])
        cT = f16s.tile([P, IC, BL], F16, tag="cT", name="cT")
        _transpose_small(nc, psT, ident16, c_f16, cT, IC)

        # --- gates (natural orientation, lhsT = transposed states) ---
        def gate_matmuls(ps, wo_name, uw, cw):
            nc.tensor.matmul(ps[:], outT_f16[:], wsmall[wo_name][:],
                             start=True, stop=False)
            for kc in range(HC):
                nc.tensor.matmul(ps[:], sT_f16[:, kc, :], uw[:, kc, :],
                                 start=False, stop=False)
            for kc in range(IC):
                nc.tensor.matmul(ps[:], cT[:, kc, :], cw[:, kc, :],
                                 start=False, stop=(kc == IC - 1))

        rps = psG.tile([BL, H], F32, tag="g", name="r_ps")
        gate_matmuls(rps, "wr", ur_sb, cr_sb)
        th_r = work.tile([BL, H], F32, tag="thr", name="th_r")
        nc.scalar.activation(th_r[:], rps[:], AF.Tanh, scale=0.5)
        # r = 0.5*th_r + 0.5 ; rs = r * s
        nc.vector.tensor_scalar(th_r[:], th_r[:], 0.5, 0.5, ALU.mult, ALU.add)
        rs = work.tile([BL, H], F32, tag="rs", name="rs")
        nc.vector.tensor_tensor(rs[:], th_r[:], s_nat[:], ALU.mult)
        rs_f16 = f16s.tile([BL, H], F16, tag="rsf16", name="rs_f16")
        nc.vector.tensor_copy(rs_f16[:], rs[:])
        rsT = f16s.tile([P, HC, BL], F16, tag="rsT", name="rsT")
        _transpose_small(nc, psT, ident16, rs_f16, rsT, HC)

        zps = psG.tile([BL, H], F32, tag="g", name="z_ps")
        gate_matmuls(zps, "wz", uz_sb, cz_sb)
        th_z = work.tile([BL, H], F32, tag="thz", name="th_z")
        nc.scalar.activation(th_z[:], zps[:], AF.Tanh, scale=0.5)

        hps = psG.tile([BL, H], F32, tag="g", name="h_ps")
        nc.tensor.matmul(hps[:], outT_f16[:], wsmall["w0"][:],
                         start=True, stop=False)
        for kc in range(HC):
            nc.tensor.matmul(hps[:], rsT[:, kc, :], u0_sb[:, kc, :],
                             start=False, stop=False)
        for kc in range(IC):
            nc.tensor.matmul(hps[:], cT[:, kc, :], c0_sb[:, kc, :],
                             start=False, stop=(kc == IC - 1))
        sh = work.tile([BL, H], F32, tag="sh", name="sh")
        nc.scalar.activation(sh[:], hps[:], AF.Tanh)

        # --- s_new = 0.5*[(s + sh) + th_z*(sh - s)] ---
        ssum = work.tile([BL, H], F32, tag="ssum", name="ssum")
        nc.vector.tensor_tensor(ssum[:], s_nat[:], sh[:], ALU.add)
        nc.vector.tensor_tensor(sh[:], sh[:], s_nat[:], ALU.subtract)
        nc.vector.tensor_tensor(sh[:], th_z[:], sh[:], ALU.mult)
        nc.vector.tensor_tensor(ssum[:], ssum[:], sh[:], ALU.add)
        s_new = state.tile([BL, H], F32, tag="s", name=f"s{k}_nat")
        nc.vector.tensor_scalar_mul(s_new[:], ssum[:], 0.5)
        s_nat = s_new

        if debug and k == 1:
            nc.sync.dma_start(io["dbg_swa"], swa_f16[:])
            e_sb = work.tile([BL, T], F32, tag="dbg_e", name="dbg_e_sb")
            nc.vector.tensor_copy(e_sb[:], e_ps[:])
            nc.sync.dma_start(io["dbg_e"], e_sb[:])
            nc.sync.dma_start(io["dbg_a"], a_f16[:])
            nc.sync.dma_start(io["dbg_c"], c_f16[:])
            r_sb = work.tile([BL, H], F32, tag="dbg_r", name="dbg_r_sb")
            nc.vector.tensor_copy(r_sb[:], rps[:])
            nc.sync.dma_start(io["dbg_rpre"], r_sb[:])
            nc.sync.dma_start(io["dbg_sh"], sh[:])
            nc.sync.dma_start(io["dbg_s"], s_new[:])
            if "dbg_U" in io:
                nc.sync.dma_start(io["dbg_U"], U_sb[:])

        # --- transposed states for next step / fc ---
        stps = psT.tile([P, HC, BL], F32, tag="tp", name="sT_ps")
        sf16 = f16s.tile([BL, H], F16, tag="sf16", name="s_f16")
        nc.vector.tensor_copy(sf16[:], s_new[:])
        for hc in range(HC):
            nc.tensor.transpose(stps[:, hc, :], s_new[:, hc * P:(hc + 1) * P],
                                ident32[:BL, :BL])
        sT_f32 = state.tile([P, HC, BL], F32, tag="sT32", name=f"s{k}T_f32")
        sT_f16 = state.tile([P, HC, BL], F16, tag="sT16", name=f"s{k}T_f16")
        nc.vector.tensor_copy(sT_f32[:], stps[:])
        nc.scalar.copy(sT_f16[:], stps[:])

        # --- out = s @ fc_w + fc_b ---
        ops = psT.tile([BL, O], F32, tag="tp", name="out_ps")
        for kc in range(HC):
            nc.tensor.matmul(ops[:], sT_f32[:, kc, :], fcw_sb[:, kc, :],
                             start=(kc == 0), stop=(kc == HC - 1))
        out_nat = state.tile([BL, O], F32, tag="out", name=f"out{k}_nat")
        nc.vector.tensor_tensor(out_nat[:], ops[:], fcb_sb[:], ALU.add)
        nc.sync.dma_start(out[k], out_nat[:])
        if k < L - 1:
            of16 = f16s.tile([BL, O], F16, tag="of16", name=f"out{k}_f16")
            nc.vector.tensor_copy(of16[:], out_nat[:])
            otp = psT.tile([O, BL], F16, tag="tp", name=f"out{k}T_ps")
            nc.tensor.transpose(otp[:], of16[:], ident16[:BL, :BL])
            outT_f16 = state.tile([P, BL], F16, tag="outT", name=f"out{k}T")
            nc.vector.memset(outT_f16[:], 0.0)
            nc.vector.tensor_copy(outT_f16[:O, :], otp[:])


_BUILT = {}


def _get_nc(L: int, debug: bool = False):
    key = (L, debug)
    if key in _BUILT:
        return _BUILT[key]
    nc = bacc.Bacc("TRN2", target_bir_lowering=False, debug=False,
                   enable_asserts=False, num_devices=NCORES)
    io = {}
    io["x"] = nc.dram_tensor("x", [T, BL, I], F32, kind="ExternalInput").ap()
    shapes = {"w0": [O, H], "wz": [O, H], "wr": [O, H], "ws": [I, H],
              "wa": [H, H], "ua": [I, H], "va": [H, 1], "u0": [H, H],
              "uz": [H, H], "ur": [H, H], "c0": [I, H], "cz": [I, H],
              "cr": [I, H], "fc_w": [H, O], "fc_b": [O]}
    for nm, shp in shapes.items():
        io[nm] = nc.dram_tensor(nm, shp, F32, kind="ExternalInput").ap()
    io["out"] = nc.dram_tensor("out", [L, BL, O], F32,
                               kind="ExternalOutput").ap()
    if debug:
        for nm, shp, dt in [("dbg_swa", [BL, H], F16), ("dbg_e", [BL, T], F32),
                            ("dbg_a", [BL, T], F16), ("dbg_c", [BL, I], F16),
                            ("dbg_rpre", [BL, H], F32), ("dbg_sh", [BL, H], F32),
                            ("dbg_s", [BL, H], F32)]:
            io[nm] = nc.dram_tensor(nm, shp, dt, kind="ExternalOutput").ap()
    with tile.TileContext(nc) as tc_:
        with ExitStack() as ctx:
            _build_decoder(ctx, tc_, L, io, debug=debug)
    nc.compile()
    _BUILT[key] = (nc, io)
    return _BUILT[key]


def kernel(**inputs) -> np.ndarray:
    L = int(np.asarray(inputs["max_labels"]))
    nc, _ = _get_nc(L)
    x = np.ascontiguousarray(np.asarray(inputs["x"], dtype=np.float32))
    base = {nm: np.ascontiguousarray(np.asarray(inputs[nm], dtype=np.float32))
            for nm in WNAMES}
    base["fc_b"] = base["fc_b"].reshape(O)
    in_maps = []
    for c in range(NCORES):
        m = dict(base)
        m["x"] = np.ascontiguousarray(x[:, c * BL:(c + 1) * BL, :])
        in_maps.append(m)
    res = run_bass_kernel_spmd(nc, in_maps, core_ids=list(range(NCORES)))
    outs = [r["out"] for r in res.results]            # each [L, BL, O]
    full = np.concatenate([o.transpose(1, 0, 2) for o in outs], axis=0)
    return np.ascontiguousarray(full.astype(np.float32))


if __name__ == "__main__":
    import reference
    ins = reference.setup_inputs()
    got = kernel(**{k: np.asarray(v) if not isinstance(v, int) else v
                    for k, v in ins.items()})
    print("kernel output", got.shape, got.dtype)


# revision 30
# speedup vs baseline: 18.8366x; 18.8366x over previous
"""Trainium2 Bass kernel for a Bahdanau-attention GRU decoder.

Reference computation (T=512, B=128, I=H=512, O=12, L=max_labels=16):
    s0 = tanh(x[0] @ ws);  out0 = s0 @ fc_w + fc_b
    U  = einsum('tbi,ih->tbh', x, ua)            # precomputed once
    per step:
        e  = einsum('tbh,h->tb', tanh(s @ wa + U), va)
        a  = softmax(e, axis=t)
        c  = einsum('tb,tbi->bi', a, x)
        r  = sigmoid(out @ wr + s @ ur + c @ cr)
        z  = sigmoid(out @ wz + s @ uz + c @ cz)
        sh = tanh(out @ w0 + (r*s) @ u0 + c @ c0)
        s  = (1-z)*s + z*sh;  out = s @ fc_w + fc_b
    returns [B, L, O]

Sharding: data-parallel over batch B across 8 cores (BL=16 per core), all
weights replicated; no collectives.  Per core, x (fp16, [i,(b,t)] natural
tiles) and U (fp16, [h-part, t, b]) are SBUF-resident so the recurrence never
touches HBM.

Per-step engine split:
  DVE : V = U + broadcast(s@wa^T)   (fp16 tensor_tensor, 2x mode, b-innermost)
  ACT : tanh(V) in-place on [128, 256*16] slabs; exp for softmax; gate tanh
        (sigmoid is computed as 0.5*tanh(x/2)+0.5 to stay in one ACT table set)
  PE  : e-dot via constant "va-selector" lhsT [128,16] (column b = va chunk)
        accumulating all b into one PSUM bank as e[b, t]; context matvecs;
        gate matmuls in natural orientation (lhsT = small transposed states);
        128x128 transposes for state/layout changes.
"""

import numpy as np
from contextlib import ExitStack

import concourse.bass as bass
import concourse.mybir as mybir
import concourse.tile as tile
from concourse import bacc
from concourse.bass_utils import run_bass_kernel_spmd
from concourse.masks import make_identity

F32 = mybir.dt.float32
F16 = mybir.dt.float16
AF = mybir.ActivationFunctionType
ALU = mybir.AluOpType
AX = mybir.AxisListType

T, B, I, H, O = 512, 128, 512, 512, 12
P = 128
NCORES = 8
BL = B // NCORES        # 16 batches per core
HC = H // P             # 4 h-chunks
IC = I // P             # 4 i-chunks
TC = T // P             # 4 t-chunks
NTH = 4                 # t-quarters for the attention slabs
THL = T // NTH          # 256

WNAMES = ["w0", "wz", "wr", "ws", "wa", "ua", "va", "u0", "uz", "ur",
          "c0", "cz", "cr", "fc_w", "fc_b"]


def _load_weight_pkh(nc, pool, wname, ap, kc, cast_pool, dtype=F16):
    """DRAM [K, H] fp32 -> SBUF [P, kc, H] in `dtype` (cast via DVE copy)."""
    w16 = pool.tile([P, kc, H], dtype, name=f"{wname}_sb")
    ap3 = ap.rearrange("(c p) h -> p c h", p=P)
    for c in range(kc):
        tmp = cast_pool.tile([P, H], F32, tag="wload", name=f"{wname}_f32tmp")
        nc.sync.dma_start(tmp[:], ap3[:, c, :])
        nc.vector.tensor_copy(w16[:, c, :], tmp[:])
    return w16


def _transpose_small(nc, psum_pool, ident16, src, dst, nchunk, tagp="tp"):
    """src [BL, nchunk*128] f16 SBUF  ->  dst [P, nchunk, BL] f16 SBUF
    via nchunk PE transposes + 1 copy."""
    ps = psum_pool.tile([P, nchunk, BL], F16, tag=tagp, name="tps")
    for c in range(nchunk):
        nc.tensor.transpose(ps[:, c, :], src[:, c * P:(c + 1) * P],
                            ident16[:BL, :BL])
    nc.vector.tensor_copy(dst[:], ps[:])


def _build_decoder(ctx: ExitStack, tc_: tile.TileContext, L: int, io: dict,
                   debug: bool = False):
    nc = tc_.nc
    x, out = io["x"], io["out"]

    const = ctx.enter_context(tc_.tile_pool(name="const", bufs=1))
    big = ctx.enter_context(tc_.tile_pool(name="big", bufs=1))

    ident16 = const.tile([P, P], F16)
    make_identity(nc, ident16[:])
    ident32 = const.tile([P, P], F32)
    make_identity(nc, ident32[:])

    # ---------------- persistent SBUF tensors ----------------
    x_nat = big.tile([P, BL, TC, I], F16)    # x[t%128, b, t//128, i]   64KB/par
    U_sb = big.tile([P, HC, T, BL], F16)     # U[h%128, h//128, t, b]   64KB/par

    # ---------------- state tiles (ping-pong via bufs=2 pools) ----------------
    state = ctx.enter_context(tc_.tile_pool(name="state", bufs=2))

    s_nat = state.tile([BL, H], F32, tag="s", name="s0_nat")
    sT_f32 = state.tile([P, HC, BL], F32, tag="sT32", name="s0T_f32")
    sT_f16 = state.tile([P, HC, BL], F16, tag="sT16", name="s0T_f16")
    out_nat = state.tile([BL, O], F32, tag="out", name="out0_nat")
    outT_f16 = state.tile([P, BL], F16, tag="outT", name="out0T_f16")

    # ---------------- setup: weights, load x, transpose, U = x @ ua, s0 ------
    with tc_.tile_pool(name="setup", bufs=2) as stp, \
         tc_.tile_pool(name="setup1", bufs=1) as stp1, \
         tc_.tile_pool(name="wcast", bufs=2) as wcast, \
         tc_.tile_pool(name="stpsA", bufs=2, space="PSUM") as stpsA, \
         tc_.tile_pool(name="stpsB", bufs=2, space="PSUM") as stpsB, \
         tc_.tile_pool(name="stpsC", bufs=1, space="PSUM") as stpsC:

        wa_sb = _load_weight_pkh(nc, const, "wa", io["wa"], HC, wcast)
        ur_sb = _load_weight_pkh(nc, const, "ur", io["ur"], HC, wcast)
        uz_sb = _load_weight_pkh(nc, const, "uz", io["uz"], HC, wcast)
        u0_sb = _load_weight_pkh(nc, const, "u0", io["u0"], HC, wcast)
        cr_sb = _load_weight_pkh(nc, const, "cr", io["cr"], IC, wcast)
        cz_sb = _load_weight_pkh(nc, const, "cz", io["cz"], IC, wcast)
        c0_sb = _load_weight_pkh(nc, const, "c0", io["c0"], IC, wcast)

        # [O, H] gate input weights, zero-padded to K=128 partitions
        # (K<128 matmuls are unreliable: the PE contracts over the full
        # partition range, so unused partitions must be zero)
        wsmall = {}
        for nm in ("wr", "wz", "w0"):
            tmp = wcast.tile([O, H], F32, tag="wsload", name=f"{nm}_f32tmp", bufs=1)
            nc.sync.dma_start(tmp[:], io[nm])
            w16 = const.tile([P, H], F16, name=f"{nm}_sb")
            nc.vector.memset(w16[:], 0.0)
            nc.vector.tensor_copy(w16[:O, :], tmp[:])
            wsmall[nm] = w16

        # fc kept fp32 for output accuracy
        fcw_sb = const.tile([P, HC, O], F32)
        nc.sync.dma_start(fcw_sb[:],
                          io["fc_w"].rearrange("(c p) o -> p c o", p=P))
        fcb_sb = const.tile([BL, O], F32)
        nc.sync.dma_start(fcb_sb[:], io["fc_b"][None, :].to_broadcast((BL, O)))

        # va -> VaSel[p, hc, b, m] = va[hc*128+p] * (m == b)
        va_f32 = const.tile([P, HC], F32)
        nc.sync.dma_start(va_f32[:],
                          io["va"][:, 0].rearrange("(c p) -> p c", p=P))
        va_f16 = const.tile([P, HC], F16)
        nc.vector.tensor_copy(va_f16[:], va_f32[:])
        vasel = const.tile([P, HC, BL, BL], F16)
        nc.vector.memset(vasel[:], 0.0)
        for hc in range(HC):
            for b in range(BL):
                nc.vector.tensor_copy(vasel[:, hc, b, b:b + 1],
                                      va_f16[:, hc:hc + 1])
        # one-hot mask used to build the per-step context selector
        bsel = const.tile([P, BL, BL], F16)
        nc.vector.memset(bsel[:], 0.0)
        one_f16 = const.tile([P, 1], F16)
        nc.vector.memset(one_f16[:], 1.0)
        for b in range(BL):
            nc.vector.tensor_copy(bsel[:, b, b:b + 1], one_f16[:])

        ua_sb = _load_weight_pkh(nc, stp1, "ua", io["ua"], IC, wcast)
        ws_sb = _load_weight_pkh(nc, stp1, "ws", io["ws"], IC, wcast)

        GB = 2  # batches per transpose group
        for g in range(BL // GB):
            xT_g = stp.tile([P, IC, GB, T], F16, tag="xTg", name="xT_g")
            for bi in range(GB):
                b = g * GB + bi
                for t_ in range(TC):
                    xdma = stp.tile([P, I], F32, tag="xdma", name="xdma")
                    nc.sync.dma_start(xdma[:], x[t_ * P:(t_ + 1) * P, b, :])
                    nc.vector.tensor_copy(x_nat[:, b, t_, :], xdma[:])
                # transpose [t,i] tiles -> xT_g[i, t]
                for ic in range(IC):
                    tps = stpsA.tile([P, T], F16, tag="xtp", name="xtp")
                    for t_ in range(TC):
                        nc.tensor.transpose(
                            tps[:, t_ * P:(t_ + 1) * P],
                            x_nat[:, b, t_, ic * P:(ic + 1) * P], ident16[:])
                    nc.vector.tensor_copy(xT_g[:, ic, bi, :], tps[:])
                # U[:, hc, :, b] = sum_ic ua[ic]^T-chunk . xT
                for hc in range(HC):
                    ups = stpsB.tile([P, T], F32, tag="ups", name="ups")
                    for ic in range(IC):
                        nc.tensor.matmul(
                            ups[:], ua_sb[:, ic, hc * P:(hc + 1) * P],
                            xT_g[:, ic, bi, :],
                            start=(ic == 0), stop=(ic == IC - 1))
                    if hc % 2 == 0:
                        nc.vector.tensor_copy(U_sb[:, hc, :, b], ups[:])
                    else:
                        nc.scalar.copy(U_sb[:, hc, :, b], ups[:])

        # ---- s0 = tanh(x0 @ ws), out0 = s0 @ fc_w + fc_b ----
        x0_f32 = stp1.tile([BL, I], F32)
        nc.sync.dma_start(x0_f32[:], x[0, :, :])
        x0_f16 = stp1.tile([BL, I], F16)
        nc.vector.tensor_copy(x0_f16[:], x0_f32[:])
        x0T = stp1.tile([P, IC, BL], F16)
        _transpose_small(nc, stpsA, ident16, x0_f16, x0T, IC, tagp="xtp")

        s0T_ps = stpsC.tile([P, HC, BL], F32, name="s0T_ps")
        for hc in range(HC):
            for ic in range(IC):
                nc.tensor.matmul(
                    s0T_ps[:, hc, :], ws_sb[:, ic, hc * P:(hc + 1) * P],
                    x0T[:, ic, :], start=(ic == 0), stop=(ic == IC - 1))
        nc.scalar.activation(sT_f16[:], s0T_ps[:], AF.Tanh)
        nc.scalar.activation(sT_f32[:], s0T_ps[:], AF.Tanh)
        # s natural
        sps = stpsB.tile([BL, H], F32, tag="s0nat", name="s0nat_ps", bufs=1)
        for hc in range(HC):
            nc.tensor.transpose(sps[:, hc * P:(hc + 1) * P],
                                sT_f32[:, hc, :], ident32[:])
        nc.vector.tensor_copy(s_nat[:], sps[:])

    # ---------------- step-loop pools (opened after setup frees SBUF) -------
    work = ctx.enter_context(tc_.tile_pool(name="work", bufs=1))
    f16s = ctx.enter_context(tc_.tile_pool(name="f16s", bufs=1))
    vpool = ctx.enter_context(tc_.tile_pool(name="vpool", bufs=4))
    psA = ctx.enter_context(tc_.tile_pool(name="psA", bufs=2, space="PSUM"))
    psT = ctx.enter_context(tc_.tile_pool(name="psT", bufs=2, space="PSUM"))
    psG = ctx.enter_context(tc_.tile_pool(name="psG", bufs=2, space="PSUM"))
    psC = ctx.enter_context(tc_.tile_pool(name="psC", bufs=2, space="PSUM"))

    # out0
    ops = psT.tile([BL, O], F32, tag="tp", name="out0_ps")
    for kc in range(HC):
        nc.tensor.matmul(ops[:], sT_f32[:, kc, :], fcw_sb[:, kc, :],
                         start=(kc == 0), stop=(kc == HC - 1))
    nc.vector.tensor_tensor(out_nat[:], ops[:], fcb_sb[:], ALU.add)
    nc.sync.dma_start(out[0], out_nat[:])
    out_f16 = f16s.tile([BL, O], F16, tag="of16", name="out0_f16")
    nc.vector.tensor_copy(out_f16[:], out_nat[:])
    otp = psT.tile([O, BL], F16, tag="tp", name="out0T_ps")
    nc.tensor.transpose(otp[:], out_f16[:], ident16[:BL, :BL])
    nc.vector.memset(outT_f16[:], 0.0)
    nc.vector.tensor_copy(outT_f16[:O, :], otp[:])

    # ---------------- decode steps ----------------
    for k in range(1, L):
        # --- sWaT[h, b] = sum_h' wa[h', h] sT[h', b]  (direct, transposed) ---
        swps = psT.tile([P, HC, BL], F32, tag="tp", name="sw_ps")
        for hc in range(HC):
            for kc in range(HC):
                nc.tensor.matmul(swps[:, hc, :],
                                 wa_sb[:, kc, hc * P:(hc + 1) * P],
                                 sT_f16[:, kc, :],
                                 start=(kc == 0), stop=(kc == HC - 1))
        swaT = f16s.tile([P, HC, BL], F16, tag="swaT", name="swaT")
        nc.vector.tensor_copy(swaT[:], swps[:])

        # --- early gate matmuls: terms that only need outT/sT ---
        rps = psG.tile([BL, H], F32, tag="g", name="r_ps")
        zps = psG.tile([BL, H], F32, tag="g", name="z_ps")
        for ps, wo in ((rps, "wr"), (zps, "wz")):
            nc.tensor.matmul(ps[:], outT_f16[:], wsmall[wo][:],
                             start=True, stop=False)
        for ps, uw in ((rps, ur_sb), (zps, uz_sb)):
            for kc in range(HC):
                nc.tensor.matmul(ps[:], sT_f16[:, kc, :], uw[:, kc, :],
                                 start=False, stop=False)

        # --- attention with online softmax + in-window context ---
        # Per t-quarter q: e_q = va . tanh(U_q + sWa); m_q/M running max;
        # p_q = exp(e_q - M); C = C*exp(M_old - M) + p_q @ x_q; S likewise.
        # (flash-attention style; moves softmax+context into the ACT window)
        M_run = None   # running max [BL, 1]
        S_run = None   # running sum of exp [BL, 1]
        Csb = work.tile([BL, I], F32, tag="Csb", name="Csb")
        for q in range(NTH):
            e_q = psA.tile([BL, THL], F32, tag="e", name=f"e_q{q}")
            for hc in range(HC):
                v = vpool.tile([P, THL, BL], F16, tag="v", name="vslab")
                nc.vector.tensor_tensor(
                    v[:], U_sb[:, hc, q * THL:(q + 1) * THL, :],
                    swaT[:, hc, None, :].to_broadcast((P, THL, BL)), ALU.add)
                nc.scalar.activation(v[:], v[:], AF.Tanh)
                for b in range(BL):
                    nc.tensor.matmul(
                        e_q[:], vasel[:, hc, b, :], v[:, :, b],
                        start=(hc == 0 and b == 0),
                        stop=(hc == HC - 1 and b == BL - 1))
            m_q = work.tile([BL, 1], F32, tag="m_q", name=f"m_q{q}", bufs=2)
            nc.vector.tensor_reduce(m_q[:], e_q[:], axis=AX.X, op=ALU.max)
            if q == 0:
                M_new = m_q
            else:
                M_new = work.tile([BL, 1], F32, tag=f"M{q % 2}",
                                  name=f"M{q}")
                nc.vector.tensor_tensor(M_new[:], M_run[:], m_q[:], ALU.max)
                # scale_old = exp(M_old - M_new)
                dM = work.tile([BL, 1], F32, tag="dM", name=f"dM{q}", bufs=2)
                nc.vector.tensor_tensor(dM[:], M_run[:], M_new[:],
                                        ALU.subtract)
                sc = work.tile([BL, 1], F32, tag="sc", name=f"sc{q}", bufs=2)
                nc.scalar.activation(sc[:], dM[:], AF.Exp)
            Mn = work.tile([BL, 1], F32, tag="Mn", name=f"Mn{q}", bufs=2)
            nc.vector.tensor_scalar_mul(Mn[:], M_new[:], -1.0)
            p_q = f16s.tile([BL, THL], F16, tag="p_q", name=f"p_q{q}", bufs=2)
            s_q = work.tile([BL, 1], F32, tag="s_q", name=f"s_q{q}", bufs=2)
            nc.scalar.activation(p_q[:], e_q[:], AF.Exp, bias=Mn[:],
                                 accum_out=s_q[:])
            # transpose p_q and build the context selector for this quarter
            TCQ = THL // P
            pT = f16s.tile([P, TCQ, BL], F16, tag="pT", name=f"pT{q}", bufs=2)
            ptp = psT.tile([P, TCQ, BL], F16, tag="tp", name=f"ptp{q}")
            for sub in range(TCQ):
                nc.tensor.transpose(ptp[:, sub, :],
                                    p_q[:, sub * P:(sub + 1) * P],
                                    ident16[:BL, :BL])
            nc.vector.tensor_copy(pT[:], ptp[:])
            asel = f16s.tile([P, TCQ, BL, BL], F16, tag="asel",
                             name=f"asel{q}", bufs=2)
            nc.vector.tensor_tensor(
                asel[:], pT[:, :, :, None].to_broadcast((P, TCQ, BL, BL)),
                bsel[:, None, :, :].to_broadcast((P, TCQ, BL, BL)), ALU.mult)
            cq = psC.tile([BL, I], F32, tag="c", name=f"c_ps{q}")
            for b in range(BL):
                for sub in range(TCQ):
                    nc.tensor.matmul(cq[:], asel[:, sub, b, :],
                                     x_nat[:, b, q * TCQ + sub, :],
                                     start=(b == 0 and sub == 0),
                                     stop=(b == BL - 1 and sub == TCQ - 1))
            if q == 0:
                nc.vector.tensor_copy(Csb[:], cq[:])
                S_new = s_q
            else:
                nc.vector.tensor_scalar_mul(Csb[:], Csb[:], sc[:])
                nc.vector.tensor_tensor(Csb[:], Csb[:], cq[:], ALU.add)
                S_new = work.tile([BL, 1], F32, tag=f"S{q % 2}",
                                  name=f"S{q}")
                nc.vector.tensor_scalar(S_new[:], S_run[:], sc[:], None,
                                        ALU.mult)
                nc.vector.tensor_tensor(S_new[:], S_new[:], s_q[:], ALU.add)
            M_run, S_run = M_new, S_new

        # c = Csb / S
        rsum = work.tile([BL, 1], F32, tag="rsum", name="rsum")
        nc.vector.reciprocal(rsum[:], S_run[:])
        c_f16 = f16s.tile([BL, I], F16, tag="c", name="c_f16")
        nc.vector.tensor_scalar(c_f16[:], Csb[:], rsum[:], None, ALU.mult)
        cT = f16s.tile([P, IC, BL], F16, tag="cT", name="cT")
        _transpose_small(nc, psT, ident16, c_f16, cT, IC)

        # --- late gate matmuls (need cT / rsT) ---
        for kc in range(IC):
            nc.tensor.matmul(rps[:], cT[:, kc, :], cr_sb[:, kc, :],
                             start=False, stop=(kc == IC - 1))
        th_r = work.tile([BL, H], F32, tag="thr", name="th_r")
        nc.scalar.activation(th_r[:], rps[:], AF.Tanh, scale=0.5)
        # rs = r*s with r = 0.5*th_r + 0.5:  rs = (0.5*th_r + 0.5) * s
        rs = work.tile([BL, H], F32, tag="thz", name="rs")
        nc.vector.tensor_scalar(rs[:], th_r[:], 0.5, 0.5, ALU.mult, ALU.add)
        rs_f16 = f16s.tile([BL, H], F16, tag="rsf16", name="rs_f16")
        nc.vector.tensor_tensor(rs_f16[:], rs[:], s_nat[:], ALU.mult)
        rsT = f16s.tile([P, HC, BL], F16, tag="rsT", name="rsT")
        _transpose_small(nc, psT, ident16, rs_f16, rsT, HC)

        for kc in range(IC):
            nc.tensor.matmul(zps[:], cT[:, kc, :], cz_sb[:, kc, :],
                             start=False, stop=(kc == IC - 1))
        th_z = work.tile([BL, H], F32, tag="thz", name="th_z")
        nc.scalar.activation(th_z[:], zps[:], AF.Tanh, scale=0.5)

        hps = psG.tile([BL, H], F32, tag="g", name="h_ps")
        nc.tensor.matmul(hps[:], outT_f16[:], wsmall["w0"][:],
                         start=True, stop=False)
        for kc in range(HC):
            nc.tensor.matmul(hps[:], rsT[:, kc, :], u0_sb[:, kc, :],
                             start=False, stop=False)
        for kc in range(IC):
            nc.tensor.matmul(hps[:], cT[:, kc, :], c0_sb[:, kc, :],
                             start=False, stop=(kc == IC - 1))
        sh = work.tile([BL, H], F32, tag="sh", name="sh")
        nc.scalar.activation(sh[:], hps[:], AF.Tanh)

        # --- s_new = 0.5*(s + sh) + (0.5*th_z)*(sh - s) ---
        ssum = work.tile([BL, H], F32, tag="thr", name="ssum")
        nc.vector.tensor_tensor(ssum[:], s_nat[:], sh[:], ALU.add)
        nc.vector.tensor_tensor(sh[:], sh[:], s_nat[:], ALU.subtract)
        nc.vector.scalar_tensor_tensor(
            out=sh[:], in0=th_z[:], scalar=0.5, in1=sh[:],
            op0=ALU.mult, op1=ALU.mult)
        s_new = state.tile([BL, H], F32, tag="s", name=f"s{k}_nat")
        nc.vector.scalar_tensor_tensor(
            out=s_new[:], in0=ssum[:], scalar=0.5, in1=sh[:],
            op0=ALU.mult, op1=ALU.add)
        s_nat = s_new

        if debug and k == 1:
            nc.sync.dma_start(io["dbg_swa"], swa_f16[:])
            e_sb = work.tile([BL, T], F32, tag="dbg_e", name="dbg_e_sb")
            nc.vector.tensor_copy(e_sb[:], e_ps[:])
            nc.sync.dma_start(io["dbg_e"], e_sb[:])
            nc.sync.dma_start(io["dbg_a"], a_f16[:])
            nc.sync.dma_start(io["dbg_c"], c_f16[:])
            r_sb = work.tile([BL, H], F32, tag="dbg_r", name="dbg_r_sb")
            nc.vector.tensor_copy(r_sb[:], rps[:])
            nc.sync.dma_start(io["dbg_rpre"], r_sb[:])
            nc.sync.dma_start(io["dbg_sh"], sh[:])
            nc.sync.dma_start(io["dbg_s"], s_new[:])
            if "dbg_U" in io:
                nc.sync.dma_start(io["dbg_U"], U_sb[:])

        # --- transposed states for next step / fc ---
        stps = psT.tile([P, HC, BL], F32, tag="tp", name="sT_ps")
        for hc in range(HC):
            nc.tensor.transpose(stps[:, hc, :], s_new[:, hc * P:(hc + 1) * P],
                                ident32[:BL, :BL])
        sT_f32 = state.tile([P, HC, BL], F32, tag="sT32", name=f"s{k}T_f32")
        sT_f16 = state.tile([P, HC, BL], F16, tag="sT16", name=f"s{k}T_f16")
        nc.vector.tensor_copy(sT_f32[:], stps[:])
        nc.scalar.copy(sT_f16[:], stps[:])

        # --- out = s @ fc_w + fc_b ---
        ops = psT.tile([BL, O], F32, tag="tp", name="out_ps")
        for kc in range(HC):
            nc.tensor.matmul(ops[:], sT_f32[:, kc, :], fcw_sb[:, kc, :],
                             start=(kc == 0), stop=(kc == HC - 1))
        out_nat = state.tile([BL, O], F32, tag="out", name=f"out{k}_nat")
        nc.vector.tensor_tensor(out_nat[:], ops[:], fcb_sb[:], ALU.add)
        nc.sync.dma_start(out[k], out_nat[:])
        if k < L - 1:
            of16 = f16s.tile([BL, O], F16, tag="of16", name=f"out{k}_f16")
            nc.vector.tensor_copy(of16[:], out_nat[:])
            otp = psT.tile([O, BL], F16, tag="tp", name=f"out{k}T_ps")
            nc.tensor.transpose(otp[:], of16[:], ident16[:BL, :BL])
            outT_f16 = state.tile([P, BL], F16, tag="outT", name=f"out{k}T")
            nc.vector.memset(outT_f16[:], 0.0)
            nc.vector.tensor_copy(outT_f16[:O, :], otp[:])


_BUILT = {}


def _get_nc(L: int, debug: bool = False):
    key = (L, debug)
    if key in _BUILT:
        return _BUILT[key]
    nc = bacc.Bacc("TRN2", target_bir_lowering=False, debug=False,
                   enable_asserts=False, num_devices=NCORES)
    io = {}
    io["x"] = nc.dram_tensor("x", [T, BL, I], F32, kind="ExternalInput").ap()
    shapes = {"w0": [O, H], "wz": [O, H], "wr": [O, H], "ws": [I, H],
              "wa": [H, H], "ua": [I, H], "va": [H, 1], "u0": [H, H],
              "uz": [H, H], "ur": [H, H], "c0": [I, H], "cz": [I, H],
              "cr": [I, H], "fc_w": [H, O], "fc_b": [O]}
    for nm, shp in shapes.items():
        io[nm] = nc.dram_tensor(nm, shp, F32, kind="ExternalInput").ap()
    io["out"] = nc.dram_tensor("out", [L, BL, O], F32,
                               kind="ExternalOutput").ap()
    if debug:
        for nm, shp, dt in [("dbg_swa", [BL, H], F16), ("dbg_e", [BL, T], F32),
                            ("dbg_a", [BL, T], F16), ("dbg_c", [BL, I], F16),
                            ("dbg_rpre", [BL, H], F32), ("dbg_sh", [BL, H], F32),
                            ("dbg_s", [BL, H], F32)]:
            io[nm] = nc.dram_tensor(nm, shp, dt, kind="ExternalOutput").ap()
    with tile.TileContext(nc) as tc_:
        with ExitStack() as ctx:
            _build_decoder(ctx, tc_, L, io, debug=debug)
    nc.compile()
    _BUILT[key] = (nc, io)
    return _BUILT[key]


def kernel(**inputs) -> np.ndarray:
    L = int(np.asarray(inputs["max_labels"]))
    nc, _ = _get_nc(L)
    x = np.ascontiguousarray(np.asarray(inputs["x"], dtype=np.float32))
    base = {nm: np.ascontiguousarray(np.asarray(inputs[nm], dtype=np.float32))
            for nm in WNAMES}
    base["fc_b"] = base["fc_b"].reshape(O)
    in_maps = []
    for c in range(NCORES):
        m = dict(base)
        m["x"] = np.ascontiguousarray(x[:, c * BL:(c + 1) * BL, :])
        in_maps.append(m)
    res = run_bass_kernel_spmd(nc, in_maps, core_ids=list(range(NCORES)))
    outs = [r["out"] for r in res.results]            # each [L, BL, O]
    full = np.concatenate([o.transpose(1, 0, 2) for o in outs], axis=0)
    return np.ascontiguousarray(full.astype(np.float32))


if __name__ == "__main__":
    import reference
    ins = reference.setup_inputs()
    got = kernel(**{k: np.asarray(v) if not isinstance(v, int) else v
                    for k, v in ins.items()})
    print("kernel output", got.shape, got.dtype)
